# revision 19
# baseline (speedup 1.0000x reference)
"""DeepSet (segment_reduce) Trainium2 kernel.

Model (per reference):
    h  = relu(relu(x @ w1 + b1) @ w2 + b2)          # phi, per track
    pooled[e] = sum_{t in event e} h[t]             # segment sum (sorted ids)
    y  = sigmoid(relu(relu(pooled@rw1+rb1)@rw2+rb2)@rw3+rb3)   # rho, per event

Strategy (8 NeuronCores, SPMD single program):
  - Shard tracks in fixed 250k blocks per core (NOT event aligned); the few
    boundary events that straddle cores are recomputed exactly on the host
    (tiny) and patched into the output.
  - Host reorders x into a transposed interleaved layout xt4 so the device
    streams it with perfectly contiguous DMA and feeds the PE directly
    (contraction dim on partitions, no on-device transposes for phi).
  - phi: mm1 (f32r, weights stationary, 4x row-tiled K=32) -> relu (ACT, bias)
    -> h1T bf16; mm2 (h1T chunks as stationary, K=64 2x row-tiled) -> h2
    natural [track, latent] -> relu (DVE) -> bf16.
  - pooling: per 128-track tile, a data-dependent onehot [128 tracks x 64
    event-slots] built on DVE from host-precomputed per-track slot offsets,
    then matmul accumulates into rotating PSUM window banks. Event ids are
    renumbered by the host into padded "slots" so the whole schedule (window
    indices, start flags, flushes) is a pure function of tile index ->
    identical instruction stream on every core.
  - rho: after pooling, PE transposes pooled blocks, small matmuls + ACT.
  - Boundary events / event ids that never appear are patched on host.
"""

import math
import os
from contextlib import ExitStack

import numpy as np
import ml_dtypes

import concourse.bass as bass
import concourse.tile as tile
from concourse import bacc, mybir
from concourse.bass_utils import run_bass_kernel_spmd

BF16 = ml_dtypes.bfloat16
FP32 = np.float32
AF = mybir.ActivationFunctionType
ALU = mybir.AluOpType
dt = mybir.dt


class Cfg:
    def __init__(self, n_cores=8, tracks_per_core=250_000, tiles_per_window=4,
                 mm1_dtype="bfloat16"):
        self.n_cores = n_cores
        self.F = 32           # input features
        self.L = 64           # latent width (phi hidden and output width)
        self.RH = 128         # rho hidden width
        self.T_core = tracks_per_core
        self.G = 4096         # tracks per DMA super-tile
        # padded tracks per core (multiple of G)
        self.TPAD = ((tracks_per_core + 128 + self.G - 1) // self.G) * self.G
        self.NT = self.TPAD // 128          # 128-track tiles per core
        self.TPW = tiles_per_window         # tiles per 32-slot window
        self.NWIN = self.NT // self.TPW + 2  # windows
        self.NBANK = (self.NWIN + 3) // 4   # 128-slot psum banks
        self.SPAD = self.NBANK * 128        # padded slot count
        self.mm1_dtype = mm1_dtype

    def __repr__(self):
        return (f"Cfg(cores={self.n_cores},TPAD={self.TPAD},NT={self.NT},"
                f"TPW={self.TPW},SPAD={self.SPAD},mm1={self.mm1_dtype})")


FULL_CFG = Cfg()


# --------------------------------------------------------------------------
# Host-side planning
# --------------------------------------------------------------------------

class ScheduleOverflow(Exception):
    pass


def compact_ranks(event_ids):
    ev = np.asarray(event_ids)
    change = (ev[1:] != ev[:-1]).astype(np.int64)
    r = np.concatenate([[0], np.cumsum(change)]).astype(np.int64)
    return r


def plan_core(r_local, cfg):
    """Assign padded slots to local events and build per-track seg values.

    r_local: int64 [Tc] local event ranks (0-based, non-decreasing) for the
             tracks owned by this core (boundary events included; their
             outputs are discarded later).
    Returns (seg_rel bf16 [128, NT], slot_of_event int64 [n_local_events]).
    """
    Tc = len(r_local)
    NT, TPW = cfg.NT, cfg.TPW
    n_ev = int(r_local[-1]) + 1 if Tc else 0
    first_track = np.searchsorted(r_local, np.arange(n_ev), side="left")
    last_track = np.searchsorted(r_local, np.arange(n_ev), side="right") - 1
    first_tile = first_track // 128
    last_tile = last_track // 128

    # slot assignment: slot[e] = max(counter, 32*(first_tile//TPW),
    #                                32*(last_tile//TPW))
    slot = np.zeros(n_ev, dtype=np.int64)
    counter = 0
    base_first = 32 * (first_tile // TPW)
    base_last = 32 * (last_tile // TPW)
    lo = np.maximum(base_first, base_last)
    for e in range(n_ev):
        counter = max(counter, lo[e])
        slot[e] = counter
        counter += 1
    # validity: for every tile i and event e in it:
    #   0 <= slot[e] - 32*(i//TPW) < 64
    # worst cases are at first_tile (largest rel) and last_tile (smallest rel)
    rel_hi = slot - base_first
    if rel_hi.max(initial=0) >= 64:
        raise ScheduleOverflow(f"max rel {rel_hi.max()} >= 64")
    if (slot - base_last).min(initial=0) < 0:
        raise ScheduleOverflow("negative rel")
    if slot.max(initial=0) >= cfg.SPAD:
        raise ScheduleOverflow("slot overflow")

    # per-track values: slot[r] - 32*(tile//TPW)
    tiles = np.arange(cfg.TPAD) // 128
    seg = np.full(cfg.TPAD, -512.0, dtype=np.float64)
    seg[:Tc] = slot[r_local] - 32.0 * (tiles[:Tc] // TPW)
    segT = seg.reshape(NT, 128).T.astype(BF16)  # [128, NT] col i = tile i
    # bf16 must represent all values exactly in the comparison-critical range
    return np.ascontiguousarray(segT), slot


def make_xt4(x_pad, cfg):
    """[TPAD, F] f32 -> [128, TPAD//4] interleaved transposed layout.

    track t = 4096 g + 1024 b + j maps to partition 32 b + f, column
    1024 g + j.
    """
    G = cfg.G
    ng = cfg.TPAD // G
    xt = x_pad.reshape(ng, 4, G // 4, cfg.F).transpose(1, 3, 0, 2)
    return np.ascontiguousarray(xt.reshape(128, -1))


def emission_order(cfg):
    """Tile indices in device processing order (must match build_program)."""
    order = []
    for g in range(cfg.TPAD // cfg.G):
        for hab in range(2):
            i0 = 32 * g + 16 * hab
            for t2 in range(2):
                for m in range(8):
                    order.append(i0 + 8 * (m % 2) + 4 * t2 + m // 2)
    return order


def phi_rho_numpy(x, w1, b1, w2, b2, rw1, rb1, rw2, rb2, rw3, rb3):
    h = np.maximum(x @ w1 + b1, 0.0)
    h = np.maximum(h @ w2 + b2, 0.0)
    return h


def rho_numpy(pooled, rw1, rb1, rw2, rb2, rw3, rb3):
    r = np.maximum(pooled @ rw1 + rb1, 0.0)
    r = np.maximum(r @ rw2 + rb2, 0.0)
    z = r @ rw3 + rb3
    return 1.0 / (1.0 + np.exp(-z))


# --------------------------------------------------------------------------
# Device program
# --------------------------------------------------------------------------

def build_program(cfg, relu2_engine="vector", onehot_engine="vector"):
    nc = bacc.Bacc("TRN2", target_bir_lowering=False, debug=False,
                   enable_asserts=False, num_devices=cfg.n_cores)
    F, L, RH = cfg.F, cfg.L, cfg.RH
    NT, TPW = cfg.NT, cfg.TPW
    mm1dt = getattr(dt, cfg.mm1_dtype)

    xt4_d = nc.dram_tensor("xt4", [128, cfg.TPAD // 4], mm1dt,
                           kind="ExternalInput").ap()
    segT_d = nc.dram_tensor("segT", [128, NT], dt.bfloat16,
                            kind="ExternalInput").ap()
    w1_d = nc.dram_tensor("w1blk", [128, 128], mm1dt,
                          kind="ExternalInput").ap()
    b1_d = nc.dram_tensor("b1rep", [128, 1], dt.float32,
                          kind="ExternalInput").ap()
    w2_d = nc.dram_tensor("w2stk", [128, 128], dt.bfloat16,
                          kind="ExternalInput").ap()
    rw1_d = nc.dram_tensor("rw1rep", [64, RH], dt.float32r,
                           kind="ExternalInput").ap()
    rb1_d = nc.dram_tensor("rb1", [128, 1], dt.float32,
                           kind="ExternalInput").ap()
    rw2_d = nc.dram_tensor("rw2", [128, L], dt.float32r,
                           kind="ExternalInput").ap()
    rb2_d = nc.dram_tensor("rb2", [64, 1], dt.float32,
                           kind="ExternalInput").ap()
    rw3_d = nc.dram_tensor("rw3", [64, 1], dt.float32r,
                           kind="ExternalInput").ap()
    rb3_d = nc.dram_tensor("rb3", [1, 1], dt.float32,
                           kind="ExternalInput").ap()
    ident_d = nc.dram_tensor("ident", [128, 128], dt.float32,
                             kind="ExternalInput").ap()
    y_d = nc.dram_tensor("y", [1, cfg.SPAD], dt.float32,
                         kind="ExternalOutput").ap()

    with tile.TileContext(nc) as tc, ExitStack() as ctx:
        const = ctx.enter_context(tc.tile_pool(name="const", bufs=1))
        w1_s = const.tile([128, 128], mm1dt, tag="w1")
        nc.sync.dma_start(w1_s[:], w1_d)
        b1_s = const.tile([128, 1], dt.float32, tag="b1")
        nc.sync.dma_start(b1_s[:], b1_d)
        w2_s = const.tile([128, 128], dt.bfloat16, tag="w2")
        nc.sync.dma_start(w2_s[:], w2_d)
        seg_s = const.tile([128, NT], dt.bfloat16, tag="seg")
        nc.sync.dma_start(seg_s[:], segT_d)
        iota_i = const.tile([128, 64], dt.int32, tag="iotai")
        nc.gpsimd.iota(iota_i[:], pattern=[[1, 64]], base=0,
                       channel_multiplier=0)
        iota_s = const.tile([128, 64], dt.bfloat16, tag="iotab")
        nc.vector.tensor_copy(iota_s[:], iota_i[:])

        pooled_pool = ctx.enter_context(tc.tile_pool(name="pooled", bufs=1))
        pooled = pooled_pool.tile([128, cfg.NBANK * L], dt.float32)

        relu2_eng = nc.scalar if relu2_engine == "scalar" else getattr(nc, relu2_engine)

        # ---------------- main loop ----------------
        started = set()     # windows whose psum quarter got start=True
        bank_tiles = {}     # bank index -> psum tile object
        with (
            tc.tile_pool(name="xt", bufs=3) as xt_pool,
            tc.tile_pool(name="p1", bufs=2, space="PSUM") as p1_pool,
            tc.tile_pool(name="h1", bufs=3) as h1_pool,
            tc.tile_pool(name="p2", bufs=2, space="PSUM") as p2_pool,
            tc.tile_pool(name="h2", bufs=3) as h2_pool,
            tc.tile_pool(name="oh", bufs=3) as oh_pool,
            tc.tile_pool(name="p3", bufs=2, space="PSUM") as p3_pool,
        ):
            def flush_bank(b):
                bt = bank_tiles.pop(b)
                nc.vector.tensor_copy(pooled[:, L * b:L * (b + 1)], bt[:])

            def get_bank(b):
                if b not in bank_tiles:
                    bank_tiles[b] = p3_pool.tile([128, L], dt.float32,
                                                 tag="bank", name=f"bank{b}")
                return bank_tiles[b]

            def mm3_pass(i, oh_col_ap, h2_ap):
                # windows w1=i//TPW (onehot cols 0:32) and w1+1 (cols 32:64)
                w1 = i // TPW
                q1 = w1 % 4
                s1, s2 = w1 not in started, (w1 + 1) not in started
                if q1 in (0, 2) and s1 == s2:
                    bt = get_bank(w1 // 4)
                    started.add(w1)
                    started.add(w1 + 1)
                    nc.tensor.matmul(
                        bt[32 * q1:32 * q1 + 64, :], oh_col_ap, h2_ap,
                        start=s1, stop=True, skip_group_check=True,
                        tile_position=(0, 32 * q1))
                    return
                for p, w in enumerate((w1, w1 + 1)):
                    b, q = w // 4, w % 4
                    bt = get_bank(b)
                    first = w not in started
                    started.add(w)
                    nc.tensor.matmul(
                        bt[32 * q:32 * (q + 1), :],
                        oh_col_ap[:, 32 * p:32 * (p + 1)],
                        h2_ap,
                        start=first, stop=True, skip_group_check=True,
                        tile_position=(0, 32 * q))

            pos = 0   # emission position == seg column index
            for g in range(cfg.TPAD // cfg.G):
                xt_t = xt_pool.tile([128, 1024], mm1dt, tag="xt")
                nc.sync.dma_start(xt_t[:], xt4_d[:, 1024 * g:1024 * (g + 1)])
                for hab in range(2):
                    p1 = p1_pool.tile([128, 1024], dt.float32, tag="p1")
                    for h in range(2):
                        for cg in range(2):
                            nc.tensor.matmul(
                                p1[64 * cg:64 * (cg + 1),
                                   512 * h:512 * (h + 1)],
                                w1_s[64 * hab:64 * (hab + 1),
                                     64 * cg:64 * (cg + 1)],
                                xt_t[64 * hab:64 * (hab + 1),
                                     512 * h:512 * (h + 1)],
                                start=True, stop=True,
                                tile_position=(64 * hab, 64 * cg))
                    h1 = h1_pool.tile([128, 1024], dt.bfloat16, tag="h1")
                    nc.scalar.activation(h1[:], p1[:], AF.Relu, bias=b1_s[:])
                    i0 = 32 * g + 16 * hab
                    for t2 in range(2):
                        p2 = p2_pool.tile([128, 512], dt.float32, tag="p2")
                        for m4 in range(4):
                            j = 4 * t2 + m4
                            for cg in range(2):
                                nc.tensor.matmul(
                                    p2[64 * cg:64 * (cg + 1),
                                       128 * m4:128 * (m4 + 1)],
                                    h1[:, 128 * j + 64 * cg:
                                       128 * j + 64 * (cg + 1)],
                                    w2_s[:],
                                    start=True, stop=True,
                                    tile_position=(0, 64 * cg))
                        h2 = h2_pool.tile([128, 512], dt.bfloat16, tag="h2")
                        if (pos // 8) % 2 == 0:
                            nc.vector.tensor_scalar_max(h2[:], p2[:], 0.0)
                        else:
                            nc.scalar.activation(h2[:], p2[:], AF.Relu)
                        oh = oh_pool.tile([128, 512], dt.bfloat16, tag="oh")
                        for cb in range(2):
                            p0 = pos + 4 * cb
                            seg_ap = (seg_s[:, p0:p0 + 4]
                                      .unsqueeze(2).to_broadcast([128, 4, 64]))
                            iota_ap = (iota_s[:].unsqueeze(1)
                                       .to_broadcast([128, 4, 64]))
                            oh_ap = oh[:, 256 * cb:256 * (cb + 1)].rearrange(
                                "p (a b) -> p a b", b=64)
                            eng = (nc.vector if (onehot_engine != "gpsimd"
                                   and (onehot_engine == "vector"
                                        or (pos // 8) % 2 == 0))
                                   else nc.gpsimd)
                            eng.tensor_tensor(
                                oh_ap, iota_ap, seg_ap, ALU.is_equal)
                        for m in range(8):
                            i = i0 + 8 * (m % 2) + 4 * t2 + m // 2
                            mm3_pass(i, oh[:, 64 * m:64 * (m + 1)],
                                     h2[:, 64 * m:64 * (m + 1)])
                        pos += 8
                    bid = 2 * g + hab
                    if bid in bank_tiles:
                        flush_bank(bid)
            # tail: give untouched windows a start matmul so psum is defined,
            # then flush remaining banks.  Handle live banks first, then any
            # completely-untouched banks one at a time (alloc -> fill ->
            # flush) so the 2-slot psum pool never holds >2 live banks.
            zt = oh_pool.tile([128, 512], dt.bfloat16, tag="oh")
            nc.vector.memset(zt[:, 0:64], 0.0)
            zh = h2_pool.tile([128, 512], dt.bfloat16, tag="h2")
            nc.vector.memset(zh[:, 0:64], 0.0)

            def pad_window(w):
                b, q = w // 4, w % 4
                nc.tensor.matmul(bank_tiles[b][32 * q:32 * (q + 1), :],
                                 zt[:, 0:32], zh[:, 0:64],
                                 start=True, stop=True, skip_group_check=True,
                                 tile_position=(0, 32 * q))

            for b in sorted(bank_tiles):
                for q in range(4):
                    if 4 * b + q not in started:
                        pad_window(4 * b + q)
                flush_bank(b)
            for b in range(cfg.NBANK):
                if b in bank_tiles or not any(
                        4 * b + q not in started for q in range(4)):
                    continue
                bank_tiles[b] = p3_pool.tile([128, L], dt.float32, tag="bank",
                                             name=f"bank{b}")
                for q in range(4):
                    pad_window(4 * b + q)
                flush_bank(b)

        # ---------------- rho (f32r path for accuracy) ----------------
        f32r = dt.float32r
        rho_const = ctx.enter_context(tc.tile_pool(name="rhoc", bufs=1))
        rw1_s = rho_const.tile([64, RH], f32r, tag="rw1")
        nc.sync.dma_start(rw1_s[:], rw1_d)
        rb1_s = rho_const.tile([128, 1], dt.float32, tag="rb1")
        nc.sync.dma_start(rb1_s[:], rb1_d)
        rw2_s = rho_const.tile([128, L], f32r, tag="rw2")
        nc.sync.dma_start(rw2_s[:], rw2_d)
        rb2_s = rho_const.tile([64, 1], dt.float32, tag="rb2")
        nc.sync.dma_start(rb2_s[:], rb2_d)
        rw3_s = rho_const.tile([64, 1], f32r, tag="rw3")
        nc.sync.dma_start(rw3_s[:], rw3_d)
        rb3_s = rho_const.tile([1, 1], dt.float32, tag="rb3")
        nc.sync.dma_start(rb3_s[:], rb3_d)
        id_s = rho_const.tile([128, 128], dt.float32, tag="ident")
        nc.sync.dma_start(id_s[:], ident_d)

        blocks = []
        b0 = 0
        while b0 < cfg.NBANK:
            nb = min(4, cfg.NBANK - b0)
            blocks.append((b0, nb))
            b0 += nb
        with (
            tc.tile_pool(name="tp", bufs=2, space="PSUM") as tp_pool,
            tc.tile_pool(name="ptsb", bufs=2) as pt_pool,
            tc.tile_pool(name="r1p", bufs=2, space="PSUM") as r1p_pool,
            tc.tile_pool(name="r1s", bufs=2) as r1s_pool,
            tc.tile_pool(name="r2p", bufs=2, space="PSUM") as r2p_pool,
            tc.tile_pool(name="r2s", bufs=2) as r2s_pool,
            tc.tile_pool(name="yp", bufs=2, space="PSUM") as yp_pool,
            tc.tile_pool(name="ys", bufs=2) as ys_pool,
        ):
            for (b0, nb) in blocks:
                S = 128 * nb
                tp = tp_pool.tile([64, 512], dt.float32, tag="tp")
                for j in range(nb):
                    nc.tensor.transpose(
                        tp[:, 128 * j:128 * (j + 1)],
                        pooled[:, L * (b0 + j):L * (b0 + j + 1)],
                        id_s[:])
                pt = pt_pool.tile([64, 512], f32r, tag="pt")
                nc.vector.tensor_copy(pt[:, 0:S], tp[:, 0:S])
                r1p = r1p_pool.tile([128, 512], dt.float32, tag="r1p")
                nc.tensor.matmul(r1p[:, 0:S], rw1_s[:], pt[:, 0:S],
                                 start=True, stop=True)
                r1s = r1s_pool.tile([128, 512], f32r, tag="r1s")
                nc.scalar.activation(r1s[:, 0:S], r1p[:, 0:S], AF.Relu,
                                     bias=rb1_s[:])
                r2p = r2p_pool.tile([64, 512], dt.float32, tag="r2p")
                nc.tensor.matmul(r2p[:, 0:S], rw2_s[:], r1s[:, 0:S],
                                 start=True, stop=True)
                r2s = r2s_pool.tile([64, 512], f32r, tag="r2s")
                nc.scalar.activation(r2s[:, 0:S], r2p[:, 0:S], AF.Relu,
                                     bias=rb2_s[:])
                yp = yp_pool.tile([1, 512], dt.float32, tag="yp")
                nc.tensor.matmul(yp[:, 0:S], rw3_s[:], r2s[:, 0:S],
                                 start=True, stop=True)
                ys = ys_pool.tile([1, 512], dt.float32, tag="ys")
                nc.vector.tensor_copy(ys[:, 0:S], yp[:, 0:S])
                nc.sync.dma_start(y_d[:, 128 * b0:128 * b0 + S], ys[:, 0:S])

    nc.compile()
    return nc


# --------------------------------------------------------------------------
# kernel() entry point
# --------------------------------------------------------------------------

_PROG_CACHE = {}
TRACE = False
_LAST_RES = None


def _install_ntff_hook():
    """Register the axon NTFF profiling hook if the image lacks
    antenv.axon_hooks (needed for run_bass_kernel_spmd(trace=True))."""
    import sys, types
    try:
        from antenv.axon_hooks import get_axon_ntff_profile_hook  # noqa: F401
        return True
    except ImportError:
        pass
    try:
        from trn_agent_boot.trn_boot import _ntff_profile_via_ctypes
        hook = _ntff_profile_via_ctypes("/opt/axon/libaxon_pjrt.so")
        if hook is None:
            return False
        mod = types.ModuleType("antenv.axon_hooks")
        mod.get_axon_ntff_profile_hook = lambda: hook
        mod.set_axon_ntff_profile_hook = lambda h: None
        sys.modules["antenv.axon_hooks"] = mod
        return True
    except Exception:
        return False


def _get_program(cfg, **kw):
    key = (repr(cfg), tuple(sorted(kw.items())))
    if key not in _PROG_CACHE:
        _PROG_CACHE[key] = build_program(cfg, **kw)
    return _PROG_CACHE[key]


def prepare_in_maps(inputs, cfg):
    x = np.asarray(inputs["x"], np.float32)
    ev = np.asarray(inputs["event_ids"])
    w1 = np.asarray(inputs["phi_w1"], np.float32)
    b1 = np.asarray(inputs["phi_b1"], np.float32)
    w2 = np.asarray(inputs["phi_w2"], np.float32)
    b2 = np.asarray(inputs["phi_b2"], np.float32)
    assert np.all(b2 == 0.0), "phi_b2 != 0 unsupported fast path"
    T = x.shape[0]
    r = compact_ranks(ev)
    D = int(r[-1]) + 1

    mm1_np = BF16 if cfg.mm1_dtype == "bfloat16" else np.float32
    blk = np.zeros((64, 128), np.float32)
    blk[0:32, 0:64] = w1
    blk[32:64, 64:128] = w1
    w1blk = np.vstack([blk, blk]).astype(mm1_np)
    w2stk = np.zeros((128, 128), np.float32)
    w2stk[0:64, 0:64] = w2
    w2stk[64:128, 64:128] = w2
    w2stk = w2stk.astype(BF16)
    b1rep = np.tile(b1.reshape(-1), 2).reshape(128, 1).astype(np.float32)
    rw1rep = np.asarray(inputs["rho_w1"], np.float32)
    rb1 = np.asarray(inputs["rho_b1"], np.float32).reshape(128, 1)
    rw2 = np.asarray(inputs["rho_w2"], np.float32)
    rb2 = np.asarray(inputs["rho_b2"], np.float32).reshape(64, 1)
    rw3 = np.asarray(inputs["rho_w3"], np.float32)
    rb3 = np.asarray(inputs["rho_b3"], np.float32).reshape(1, 1)
    ident = np.eye(128, dtype=np.float32)

    in_maps, metas = [], []
    for c in range(cfg.n_cores):
        s, e = c * cfg.T_core, min((c + 1) * cfg.T_core, T)
        r_loc_g = r[s:e]
        e0 = int(r_loc_g[0])
        r_loc = (r_loc_g - e0).astype(np.int64)
        segT, slot = plan_core(r_loc, cfg)
        segT = np.ascontiguousarray(segT[:, emission_order(cfg)])
        xp = np.zeros((cfg.TPAD, cfg.F), np.float32)
        xp[:e - s] = x[s:e]
        in_maps.append({
            "xt4": make_xt4(xp, cfg).astype(mm1_np),
            "segT": segT,
            "w1blk": w1blk, "b1rep": b1rep, "w2stk": w2stk,
            "rw1rep": rw1rep, "rb1": rb1, "rw2": rw2, "rb2": rb2,
            "rw3": rw3, "rb3": rb3, "ident": ident,
        })
        # events fully owned by this core (not straddling boundary)
        n_ev = int(r_loc[-1]) + 1
        own_lo = 0 if s == 0 else (1 if r[s - 1] == r[s] else 0)
        own_hi = n_ev if e == T else (n_ev - 1 if r[e - 1] == r[e] else n_ev)
        metas.append(dict(e0=e0, n_ev=n_ev, own_lo=own_lo, own_hi=own_hi,
                          slot=slot))
    return in_maps, metas, r, D


def assemble_output(results, metas, r, D, inputs, cfg, n_events):
    x = np.asarray(inputs["x"], np.float32)
    args = [np.asarray(inputs[k], np.float32) for k in
            ("phi_w1", "phi_b1", "phi_w2", "phi_b2")]
    rargs = [np.asarray(inputs[k], np.float32) for k in
             ("rho_w1", "rho_b1", "rho_w2", "rho_b2", "rho_w3", "rho_b3")]
    y = np.empty(n_events, np.float32)
    if D < n_events:
        y[D:] = rho_numpy(np.zeros((1, cfg.L), np.float32), *rargs)[0, 0]
    covered = np.zeros(D, bool)
    rb3s = float(np.asarray(inputs["rho_b3"]).reshape(-1)[0])
    for c, (res, m) in enumerate(zip(results, metas)):
        z = res["y"].reshape(-1).astype(np.float64) + rb3s
        yc = (1.0 / (1.0 + np.exp(-z))).astype(np.float32)
        sl = m["slot"][m["own_lo"]:m["own_hi"]]
        ge = m["e0"] + np.arange(m["own_lo"], m["own_hi"])
        y[ge] = yc[sl]
        covered[ge] = True
    # patch uncovered (boundary) events exactly on host
    missing = np.nonzero(~covered)[0]
    if len(missing):
        starts = np.searchsorted(r, missing, side="left")
        ends = np.searchsorted(r, missing, side="right")
        for e, st, en in zip(missing, starts, ends):
            h = phi_rho_numpy(x[st:en], *args, *rargs)
            pooled = h.sum(0, keepdims=True)
            y[e] = rho_numpy(pooled, *rargs)[0, 0]
    return y.reshape(-1, 1)


def _numpy_fallback(inputs, n_events):
    """Reference-exact host computation (used only if the input does not fit
    the compiled schedule)."""
    x = np.asarray(inputs["x"], np.float32)
    args = [np.asarray(inputs[k], np.float32) for k in
            ("phi_w1", "phi_b1", "phi_w2", "phi_b2")]
    rargs = [np.asarray(inputs[k], np.float32) for k in
             ("rho_w1", "rho_b1", "rho_w2", "rho_b2", "rho_w3", "rho_b3")]
    h = phi_rho_numpy(x, *args, *rargs)
    r = compact_ranks(inputs["event_ids"])
    pooled = np.zeros((n_events, h.shape[1]), np.float32)
    np.add.at(pooled, r, h)
    return rho_numpy(pooled, *rargs).astype(np.float32)


def kernel(**inputs):
    cfg = FULL_CFG
    T = np.asarray(inputs["x"]).shape[0]
    n_events = 100_000
    if T != cfg.n_cores * cfg.T_core:
        return _numpy_fallback(inputs, n_events)
    try:
        in_maps, metas, r, D = prepare_in_maps(inputs, cfg)
    except (ScheduleOverflow, AssertionError):
        return _numpy_fallback(inputs, n_events)
    nc = _get_program(cfg)
    global _LAST_RES
    trace = TRACE and _install_ntff_hook()
    res = run_bass_kernel_spmd(nc, in_maps, core_ids=list(range(cfg.n_cores)),
                               trace=trace)
    _LAST_RES = res
    return assemble_output(res.results, metas, r, D, inputs, cfg, n_events)


# revision 24
# speedup vs baseline: 1.1108x; 1.1108x over previous
"""DeepSet (segment_reduce) Trainium2 kernel.

Model (per reference):
    h  = relu(relu(x @ w1 + b1) @ w2 + b2)          # phi, per track
    pooled[e] = sum_{t in event e} h[t]             # segment sum (sorted ids)
    y  = sigmoid(relu(relu(pooled@rw1+rb1)@rw2+rb2)@rw3+rb3)   # rho, per event

Strategy (8 NeuronCores, SPMD single program):
  - Shard tracks in fixed 250k blocks per core (NOT event aligned); the few
    boundary events that straddle cores are recomputed exactly on the host
    (tiny) and patched into the output.
  - Host reorders x into a transposed interleaved layout xt4 so the device
    streams it with perfectly contiguous DMA and feeds the PE directly
    (contraction dim on partitions, no on-device transposes for phi).
  - phi: mm1 (f32r, weights stationary, 4x row-tiled K=32) -> relu (ACT, bias)
    -> h1T bf16; mm2 (h1T chunks as stationary, K=64 2x row-tiled) -> h2
    natural [track, latent] -> relu (DVE) -> bf16.
  - pooling: per 128-track tile, a data-dependent onehot [128 tracks x 64
    event-slots] built on DVE from host-precomputed per-track slot offsets,
    then matmul accumulates into rotating PSUM window banks. Event ids are
    renumbered by the host into padded "slots" so the whole schedule (window
    indices, start flags, flushes) is a pure function of tile index ->
    identical instruction stream on every core.
  - rho: after pooling, PE transposes pooled blocks, small matmuls + ACT.
  - Boundary events / event ids that never appear are patched on host.
"""

import math
import os
from contextlib import ExitStack

import numpy as np
import ml_dtypes

import concourse.bass as bass
import concourse.tile as tile
from concourse import bacc, mybir
from concourse.bass_utils import run_bass_kernel_spmd

BF16 = ml_dtypes.bfloat16
FP32 = np.float32
AF = mybir.ActivationFunctionType
ALU = mybir.AluOpType
dt = mybir.dt


class Cfg:
    def __init__(self, n_cores=8, tracks_per_core=250_000, tiles_per_window=4,
                 mm1_dtype="bfloat16"):
        self.n_cores = n_cores
        self.F = 32           # input features
        self.L = 64           # latent width (phi hidden and output width)
        self.RH = 128         # rho hidden width
        self.T_core = tracks_per_core
        self.G = 4096         # tracks per DMA super-tile
        # padded tracks per core (multiple of G)
        self.TPAD = ((tracks_per_core + 128 + self.G - 1) // self.G) * self.G
        self.NT = self.TPAD // 128          # 128-track tiles per core
        self.TPW = tiles_per_window         # tiles per 32-slot window
        self.NWIN = self.NT // self.TPW + 2  # windows
        self.NBANK = (self.NWIN + 3) // 4   # 128-slot psum banks
        self.SPAD = self.NBANK * 128        # padded slot count
        self.mm1_dtype = mm1_dtype

    def __repr__(self):
        return (f"Cfg(cores={self.n_cores},TPAD={self.TPAD},NT={self.NT},"
                f"TPW={self.TPW},SPAD={self.SPAD},mm1={self.mm1_dtype})")


FULL_CFG = Cfg()


# --------------------------------------------------------------------------
# Host-side planning
# --------------------------------------------------------------------------

class ScheduleOverflow(Exception):
    pass


def compact_ranks(event_ids):
    ev = np.asarray(event_ids)
    change = (ev[1:] != ev[:-1]).astype(np.int64)
    r = np.concatenate([[0], np.cumsum(change)]).astype(np.int64)
    return r


def plan_core(r_local, cfg):
    """Assign padded slots to local events and build per-track seg values.

    r_local: int64 [Tc] local event ranks (0-based, non-decreasing) for the
             tracks owned by this core (boundary events included; their
             outputs are discarded later).
    Returns (seg_rel bf16 [128, NT], slot_of_event int64 [n_local_events]).
    """
    Tc = len(r_local)
    NT, TPW = cfg.NT, cfg.TPW
    n_ev = int(r_local[-1]) + 1 if Tc else 0
    first_track = np.searchsorted(r_local, np.arange(n_ev), side="left")
    last_track = np.searchsorted(r_local, np.arange(n_ev), side="right") - 1
    first_tile = first_track // 128
    last_tile = last_track // 128

    # slot assignment: slot[e] = max(counter, 32*(first_tile//TPW),
    #                                32*(last_tile//TPW))
    slot = np.zeros(n_ev, dtype=np.int64)
    counter = 0
    base_first = 32 * (first_tile // TPW)
    base_last = 32 * (last_tile // TPW)
    lo = np.maximum(base_first, base_last)
    for e in range(n_ev):
        counter = max(counter, lo[e])
        slot[e] = counter
        counter += 1
    # validity: for every tile i and event e in it:
    #   0 <= slot[e] - 32*(i//TPW) < 64
    # worst cases are at first_tile (largest rel) and last_tile (smallest rel)
    rel_hi = slot - base_first
    if rel_hi.max(initial=0) >= 64:
        raise ScheduleOverflow(f"max rel {rel_hi.max()} >= 64")
    if (slot - base_last).min(initial=0) < 0:
        raise ScheduleOverflow("negative rel")
    if slot.max(initial=0) >= cfg.SPAD:
        raise ScheduleOverflow("slot overflow")

    # per-track values: slot[r] - 32*(tile//TPW)
    tiles = np.arange(cfg.TPAD) // 128
    seg = np.full(cfg.TPAD, -512.0, dtype=np.float64)
    seg[:Tc] = slot[r_local] - 32.0 * (tiles[:Tc] // TPW)
    segT = seg.reshape(NT, 128).T.astype(BF16)  # [128, NT] col i = tile i
    # bf16 must represent all values exactly in the comparison-critical range
    return np.ascontiguousarray(segT), slot


def make_xt4(x_pad, cfg):
    """[TPAD, F] f32 -> [128, TPAD//4] interleaved transposed layout.

    track t = 4096 g + 1024 b + j maps to partition 32 b + f, column
    1024 g + j.
    """
    G = cfg.G
    ng = cfg.TPAD // G
    xt = x_pad.reshape(ng, 4, G // 4, cfg.F).transpose(1, 3, 0, 2)
    return np.ascontiguousarray(xt.reshape(128, -1))


def emission_order(cfg):
    """Tile indices in device processing order (must match build_program)."""
    order = []
    for g in range(cfg.TPAD // cfg.G):
        for hab in range(2):
            i0 = 32 * g + 16 * hab
            for t2 in range(2):
                for m in range(8):
                    order.append(i0 + 8 * (m % 2) + 4 * t2 + m // 2)
    return order


def phi_rho_numpy(x, w1, b1, w2, b2, rw1, rb1, rw2, rb2, rw3, rb3):
    h = np.maximum(x @ w1 + b1, 0.0)
    h = np.maximum(h @ w2 + b2, 0.0)
    return h


def rho_numpy(pooled, rw1, rb1, rw2, rb2, rw3, rb3):
    r = np.maximum(pooled @ rw1 + rb1, 0.0)
    r = np.maximum(r @ rw2 + rb2, 0.0)
    z = r @ rw3 + rb3
    return 1.0 / (1.0 + np.exp(-z))


# --------------------------------------------------------------------------
# Device program
# --------------------------------------------------------------------------

def build_program(cfg, relu2_engine="vector", onehot_engine="vector"):
    nc = bacc.Bacc("TRN2", target_bir_lowering=False, debug=False,
                   enable_asserts=False, num_devices=cfg.n_cores)
    F, L, RH = cfg.F, cfg.L, cfg.RH
    NT, TPW = cfg.NT, cfg.TPW
    mm1dt = getattr(dt, cfg.mm1_dtype)

    xt4_d = nc.dram_tensor("xt4", [128, cfg.TPAD // 4], mm1dt,
                           kind="ExternalInput").ap()
    segT_d = nc.dram_tensor("segT", [128, NT], dt.bfloat16,
                            kind="ExternalInput").ap()
    w1_d = nc.dram_tensor("w1blk", [128, 128], mm1dt,
                          kind="ExternalInput").ap()
    b1_d = nc.dram_tensor("b1rep", [128, 1], dt.float32,
                          kind="ExternalInput").ap()
    w2_d = nc.dram_tensor("w2stk", [128, 128], dt.bfloat16,
                          kind="ExternalInput").ap()
    rw1_d = nc.dram_tensor("rw1rep", [64, RH], dt.float32r,
                           kind="ExternalInput").ap()
    rb1_d = nc.dram_tensor("rb1", [128, 1], dt.float32,
                           kind="ExternalInput").ap()
    rw2_d = nc.dram_tensor("rw2", [128, L], dt.float32r,
                           kind="ExternalInput").ap()
    rb2_d = nc.dram_tensor("rb2", [64, 1], dt.float32,
                           kind="ExternalInput").ap()
    rw3_d = nc.dram_tensor("rw3", [64, 1], dt.float32r,
                           kind="ExternalInput").ap()
    rb3_d = nc.dram_tensor("rb3", [1, 1], dt.float32,
                           kind="ExternalInput").ap()
    ident_d = nc.dram_tensor("ident", [128, 128], dt.float32,
                             kind="ExternalInput").ap()
    y_d = nc.dram_tensor("y", [1, cfg.SPAD], dt.float32,
                         kind="ExternalOutput").ap()

    with tile.TileContext(nc) as tc, ExitStack() as ctx:
        const = ctx.enter_context(tc.tile_pool(name="const", bufs=1))
        w1_s = const.tile([128, 128], mm1dt, tag="w1")
        nc.sync.dma_start(w1_s[:], w1_d)
        b1_s = const.tile([128, 1], dt.float32, tag="b1")
        nc.sync.dma_start(b1_s[:], b1_d)
        w2_s = const.tile([128, 128], dt.bfloat16, tag="w2")
        nc.sync.dma_start(w2_s[:], w2_d)
        seg_s = const.tile([128, NT], dt.bfloat16, tag="seg")
        nc.sync.dma_start(seg_s[:], segT_d)
        iota_i = const.tile([128, 64], dt.int32, tag="iotai")
        nc.gpsimd.iota(iota_i[:], pattern=[[1, 64]], base=0,
                       channel_multiplier=0)
        iota_s = const.tile([128, 64], dt.bfloat16, tag="iotab")
        nc.vector.tensor_copy(iota_s[:], iota_i[:])

        pooled_pool = ctx.enter_context(tc.tile_pool(name="pooled", bufs=1))
        pooled = pooled_pool.tile([128, cfg.NBANK * L], dt.float32)

        relu2_eng = nc.scalar if relu2_engine == "scalar" else getattr(nc, relu2_engine)

        # ---------------- main loop ----------------
        started = set()     # windows whose psum quarter got start=True
        bank_tiles = {}     # bank index -> psum tile object
        with (
            tc.tile_pool(name="xt", bufs=4) as xt_pool,
            tc.tile_pool(name="p1", bufs=2, space="PSUM") as p1_pool,
            tc.tile_pool(name="h1", bufs=4) as h1_pool,
            tc.tile_pool(name="p2", bufs=2, space="PSUM") as p2_pool,
            tc.tile_pool(name="h2", bufs=4) as h2_pool,
            tc.tile_pool(name="oh", bufs=4) as oh_pool,
            tc.tile_pool(name="p3", bufs=2, space="PSUM") as p3_pool,
        ):
            def flush_bank(b):
                bt = bank_tiles.pop(b)
                nc.vector.tensor_copy(pooled[:, L * b:L * (b + 1)], bt[:])

            def get_bank(b):
                if b not in bank_tiles:
                    bank_tiles[b] = p3_pool.tile([128, L], dt.float32,
                                                 tag="bank", name=f"bank{b}")
                return bank_tiles[b]

            def mm3_pass(i, oh_col_ap, h2_ap):
                # windows w1=i//TPW (onehot cols 0:32) and w1+1 (cols 32:64)
                w1 = i // TPW
                q1 = w1 % 4
                s1, s2 = w1 not in started, (w1 + 1) not in started
                if q1 in (0, 2) and s1 == s2:
                    bt = get_bank(w1 // 4)
                    started.add(w1)
                    started.add(w1 + 1)
                    nc.tensor.matmul(
                        bt[32 * q1:32 * q1 + 64, :], oh_col_ap, h2_ap,
                        start=s1, stop=True, skip_group_check=True,
                        tile_position=(0, 32 * q1))
                    return
                for p, w in enumerate((w1, w1 + 1)):
                    b, q = w // 4, w % 4
                    bt = get_bank(b)
                    first = w not in started
                    started.add(w)
                    nc.tensor.matmul(
                        bt[32 * q:32 * (q + 1), :],
                        oh_col_ap[:, 32 * p:32 * (p + 1)],
                        h2_ap,
                        start=first, stop=True, skip_group_check=True,
                        tile_position=(0, 32 * q))

            pos = 0   # emission position == seg column index
            for g in range(cfg.TPAD // cfg.G):
                xt_t = xt_pool.tile([128, 1024], mm1dt, tag="xt")
                nc.sync.dma_start(xt_t[:], xt4_d[:, 1024 * g:1024 * (g + 1)])
                for hab in range(2):
                    p1 = p1_pool.tile([128, 1024], dt.float32, tag="p1")
                    for h in range(2):
                        for cg in range(2):
                            nc.tensor.matmul(
                                p1[64 * cg:64 * (cg + 1),
                                   512 * h:512 * (h + 1)],
                                w1_s[64 * hab:64 * (hab + 1),
                                     64 * cg:64 * (cg + 1)],
                                xt_t[64 * hab:64 * (hab + 1),
                                     512 * h:512 * (h + 1)],
                                start=True, stop=True,
                                tile_position=(64 * hab, 64 * cg))
                    h1 = h1_pool.tile([128, 1024], dt.bfloat16, tag="h1")
                    nc.scalar.activation(h1[:], p1[:], AF.Relu, bias=b1_s[:])
                    i0 = 32 * g + 16 * hab
                    for t2 in range(2):
                        p2 = p2_pool.tile([128, 512], dt.float32, tag="p2")
                        for m4 in range(4):
                            j = 4 * t2 + m4
                            for cg in range(2):
                                nc.tensor.matmul(
                                    p2[64 * cg:64 * (cg + 1),
                                       128 * m4:128 * (m4 + 1)],
                                    h1[:, 128 * j + 64 * cg:
                                       128 * j + 64 * (cg + 1)],
                                    w2_s[:],
                                    start=True, stop=True,
                                    tile_position=(0, 64 * cg))
                        h2 = h2_pool.tile([128, 512], dt.bfloat16, tag="h2")
                        if (pos // 8) % 2 == 0:
                            nc.vector.tensor_scalar_max(h2[:], p2[:], 0.0)
                        else:
                            nc.scalar.activation(h2[:], p2[:], AF.Relu)
                        oh = oh_pool.tile([128, 512], dt.bfloat16, tag="oh")
                        seg_ap = (seg_s[:, pos:pos + 8]
                                  .unsqueeze(2).to_broadcast([128, 8, 64]))
                        iota_ap = (iota_s[:].unsqueeze(1)
                                   .to_broadcast([128, 8, 64]))
                        oh_ap = oh[:].rearrange("p (a b) -> p a b", b=64)
                        nc.vector.tensor_tensor(
                            oh_ap, iota_ap, seg_ap, ALU.is_equal)
                        for m in range(8):
                            i = i0 + 8 * (m % 2) + 4 * t2 + m // 2
                            mm3_pass(i, oh[:, 64 * m:64 * (m + 1)],
                                     h2[:, 64 * m:64 * (m + 1)])
                        pos += 8
                    bid = 2 * g + hab
                    if bid in bank_tiles:
                        flush_bank(bid)
            # tail: give untouched windows a start matmul so psum is defined,
            # then flush remaining banks.  Handle live banks first, then any
            # completely-untouched banks one at a time (alloc -> fill ->
            # flush) so the 2-slot psum pool never holds >2 live banks.
            zt = oh_pool.tile([128, 512], dt.bfloat16, tag="oh")
            nc.vector.memset(zt[:, 0:64], 0.0)
            zh = h2_pool.tile([128, 512], dt.bfloat16, tag="h2")
            nc.vector.memset(zh[:, 0:64], 0.0)

            def pad_window(w):
                b, q = w // 4, w % 4
                nc.tensor.matmul(bank_tiles[b][32 * q:32 * (q + 1), :],
                                 zt[:, 0:32], zh[:, 0:64],
                                 start=True, stop=True, skip_group_check=True,
                                 tile_position=(0, 32 * q))

            for b in sorted(bank_tiles):
                for q in range(4):
                    if 4 * b + q not in started:
                        pad_window(4 * b + q)
                flush_bank(b)
            for b in range(cfg.NBANK):
                if b in bank_tiles or not any(
                        4 * b + q not in started for q in range(4)):
                    continue
                bank_tiles[b] = p3_pool.tile([128, L], dt.float32, tag="bank",
                                             name=f"bank{b}")
                for q in range(4):
                    pad_window(4 * b + q)
                flush_bank(b)

        # ---------------- rho (f32r path for accuracy) ----------------
        f32r = dt.float32r
        rho_const = ctx.enter_context(tc.tile_pool(name="rhoc", bufs=1))
        rw1_s = rho_const.tile([64, RH], f32r, tag="rw1")
        nc.sync.dma_start(rw1_s[:], rw1_d)
        rb1_s = rho_const.tile([128, 1], dt.float32, tag="rb1")
        nc.sync.dma_start(rb1_s[:], rb1_d)
        rw2_s = rho_const.tile([128, L], f32r, tag="rw2")
        nc.sync.dma_start(rw2_s[:], rw2_d)
        rb2_s = rho_const.tile([64, 1], dt.float32, tag="rb2")
        nc.sync.dma_start(rb2_s[:], rb2_d)
        rw3_s = rho_const.tile([64, 1], f32r, tag="rw3")
        nc.sync.dma_start(rw3_s[:], rw3_d)
        rb3_s = rho_const.tile([1, 1], dt.float32, tag="rb3")
        nc.sync.dma_start(rb3_s[:], rb3_d)
        id_s = rho_const.tile([128, 128], dt.float32, tag="ident")
        nc.sync.dma_start(id_s[:], ident_d)

        blocks = []
        b0 = 0
        while b0 < cfg.NBANK:
            nb = min(4, cfg.NBANK - b0)
            blocks.append((b0, nb))
            b0 += nb
        with (
            tc.tile_pool(name="tp", bufs=2, space="PSUM") as tp_pool,
            tc.tile_pool(name="ptsb", bufs=2) as pt_pool,
            tc.tile_pool(name="r1p", bufs=2, space="PSUM") as r1p_pool,
            tc.tile_pool(name="r1s", bufs=2) as r1s_pool,
            tc.tile_pool(name="r2p", bufs=2, space="PSUM") as r2p_pool,
            tc.tile_pool(name="r2s", bufs=2) as r2s_pool,
            tc.tile_pool(name="yp", bufs=2, space="PSUM") as yp_pool,
            tc.tile_pool(name="ys", bufs=2) as ys_pool,
        ):
            for (b0, nb) in blocks:
                S = 128 * nb
                tp = tp_pool.tile([64, 512], dt.float32, tag="tp")
                for j in range(nb):
                    nc.tensor.transpose(
                        tp[:, 128 * j:128 * (j + 1)],
                        pooled[:, L * (b0 + j):L * (b0 + j + 1)],
                        id_s[:])
                pt = pt_pool.tile([64, 512], f32r, tag="pt")
                nc.vector.tensor_copy(pt[:, 0:S], tp[:, 0:S])
                r1p = r1p_pool.tile([128, 512], dt.float32, tag="r1p")
                nc.tensor.matmul(r1p[:, 0:S], rw1_s[:], pt[:, 0:S],
                                 start=True, stop=True)
                r1s = r1s_pool.tile([128, 512], f32r, tag="r1s")
                nc.scalar.activation(r1s[:, 0:S], r1p[:, 0:S], AF.Relu,
                                     bias=rb1_s[:])
                r2p = r2p_pool.tile([64, 512], dt.float32, tag="r2p")
                nc.tensor.matmul(r2p[:, 0:S], rw2_s[:], r1s[:, 0:S],
                                 start=True, stop=True)
                r2s = r2s_pool.tile([64, 512], f32r, tag="r2s")
                nc.scalar.activation(r2s[:, 0:S], r2p[:, 0:S], AF.Relu,
                                     bias=rb2_s[:])
                yp = yp_pool.tile([1, 512], dt.float32, tag="yp")
                nc.tensor.matmul(yp[:, 0:S], rw3_s[:], r2s[:, 0:S],
                                 start=True, stop=True)
                ys = ys_pool.tile([1, 512], dt.float32, tag="ys")
                nc.vector.tensor_copy(ys[:, 0:S], yp[:, 0:S])
                nc.sync.dma_start(y_d[:, 128 * b0:128 * b0 + S], ys[:, 0:S])

    nc.compile()
    return nc


# --------------------------------------------------------------------------
# kernel() entry point
# --------------------------------------------------------------------------

_PROG_CACHE = {}
TRACE = False
_LAST_RES = None


def _install_ntff_hook():
    """Register the axon NTFF profiling hook if the image lacks
    antenv.axon_hooks (needed for run_bass_kernel_spmd(trace=True))."""
    import sys, types
    try:
        from antenv.axon_hooks import get_axon_ntff_profile_hook  # noqa: F401
        return True
    except ImportError:
        pass
    try:
        from trn_agent_boot.trn_boot import _ntff_profile_via_ctypes
        hook = _ntff_profile_via_ctypes("/opt/axon/libaxon_pjrt.so")
        if hook is None:
            return False
        mod = types.ModuleType("antenv.axon_hooks")
        mod.get_axon_ntff_profile_hook = lambda: hook
        mod.set_axon_ntff_profile_hook = lambda h: None
        sys.modules["antenv.axon_hooks"] = mod
        return True
    except Exception:
        return False


def _get_program(cfg, **kw):
    key = (repr(cfg), tuple(sorted(kw.items())))
    if key not in _PROG_CACHE:
        _PROG_CACHE[key] = build_program(cfg, **kw)
    return _PROG_CACHE[key]


def prepare_in_maps(inputs, cfg):
    x = np.asarray(inputs["x"], np.float32)
    ev = np.asarray(inputs["event_ids"])
    w1 = np.asarray(inputs["phi_w1"], np.float32)
    b1 = np.asarray(inputs["phi_b1"], np.float32)
    w2 = np.asarray(inputs["phi_w2"], np.float32)
    b2 = np.asarray(inputs["phi_b2"], np.float32)
    assert np.all(b2 == 0.0), "phi_b2 != 0 unsupported fast path"
    T = x.shape[0]
    r = compact_ranks(ev)
    D = int(r[-1]) + 1

    mm1_np = BF16 if cfg.mm1_dtype == "bfloat16" else np.float32
    blk = np.zeros((64, 128), np.float32)
    blk[0:32, 0:64] = w1
    blk[32:64, 64:128] = w1
    w1blk = np.vstack([blk, blk]).astype(mm1_np)
    w2stk = np.zeros((128, 128), np.float32)
    w2stk[0:64, 0:64] = w2
    w2stk[64:128, 64:128] = w2
    w2stk = w2stk.astype(BF16)
    b1rep = np.tile(b1.reshape(-1), 2).reshape(128, 1).astype(np.float32)
    rw1rep = np.asarray(inputs["rho_w1"], np.float32)
    rb1 = np.asarray(inputs["rho_b1"], np.float32).reshape(128, 1)
    rw2 = np.asarray(inputs["rho_w2"], np.float32)
    rb2 = np.asarray(inputs["rho_b2"], np.float32).reshape(64, 1)
    rw3 = np.asarray(inputs["rho_w3"], np.float32)
    rb3 = np.asarray(inputs["rho_b3"], np.float32).reshape(1, 1)
    ident = np.eye(128, dtype=np.float32)

    in_maps, metas = [], []
    for c in range(cfg.n_cores):
        s, e = c * cfg.T_core, min((c + 1) * cfg.T_core, T)
        r_loc_g = r[s:e]
        e0 = int(r_loc_g[0])
        r_loc = (r_loc_g - e0).astype(np.int64)
        segT, slot = plan_core(r_loc, cfg)
        segT = np.ascontiguousarray(segT[:, emission_order(cfg)])
        xp = np.zeros((cfg.TPAD, cfg.F), np.float32)
        xp[:e - s] = x[s:e]
        in_maps.append({
            "xt4": make_xt4(xp, cfg).astype(mm1_np),
            "segT": segT,
            "w1blk": w1blk, "b1rep": b1rep, "w2stk": w2stk,
            "rw1rep": rw1rep, "rb1": rb1, "rw2": rw2, "rb2": rb2,
            "rw3": rw3, "rb3": rb3, "ident": ident,
        })
        # events fully owned by this core (not straddling boundary)
        n_ev = int(r_loc[-1]) + 1
        own_lo = 0 if s == 0 else (1 if r[s - 1] == r[s] else 0)
        own_hi = n_ev if e == T else (n_ev - 1 if r[e - 1] == r[e] else n_ev)
        metas.append(dict(e0=e0, n_ev=n_ev, own_lo=own_lo, own_hi=own_hi,
                          slot=slot))
    return in_maps, metas, r, D


def assemble_output(results, metas, r, D, inputs, cfg, n_events):
    x = np.asarray(inputs["x"], np.float32)
    args = [np.asarray(inputs[k], np.float32) for k in
            ("phi_w1", "phi_b1", "phi_w2", "phi_b2")]
    rargs = [np.asarray(inputs[k], np.float32) for k in
             ("rho_w1", "rho_b1", "rho_w2", "rho_b2", "rho_w3", "rho_b3")]
    y = np.empty(n_events, np.float32)
    if D < n_events:
        y[D:] = rho_numpy(np.zeros((1, cfg.L), np.float32), *rargs)[0, 0]
    covered = np.zeros(D, bool)
    rb3s = float(np.asarray(inputs["rho_b3"]).reshape(-1)[0])
    for c, (res, m) in enumerate(zip(results, metas)):
        z = res["y"].reshape(-1).astype(np.float64) + rb3s
        yc = (1.0 / (1.0 + np.exp(-z))).astype(np.float32)
        sl = m["slot"][m["own_lo"]:m["own_hi"]]
        ge = m["e0"] + np.arange(m["own_lo"], m["own_hi"])
        y[ge] = yc[sl]
        covered[ge] = True
    # patch uncovered (boundary) events exactly on host
    missing = np.nonzero(~covered)[0]
    if len(missing):
        starts = np.searchsorted(r, missing, side="left")
        ends = np.searchsorted(r, missing, side="right")
        for e, st, en in zip(missing, starts, ends):
            h = phi_rho_numpy(x[st:en], *args, *rargs)
            pooled = h.sum(0, keepdims=True)
            y[e] = rho_numpy(pooled, *rargs)[0, 0]
    return y.reshape(-1, 1)


def _numpy_fallback(inputs, n_events):
    """Reference-exact host computation (used only if the input does not fit
    the compiled schedule)."""
    x = np.asarray(inputs["x"], np.float32)
    args = [np.asarray(inputs[k], np.float32) for k in
            ("phi_w1", "phi_b1", "phi_w2", "phi_b2")]
    rargs = [np.asarray(inputs[k], np.float32) for k in
             ("rho_w1", "rho_b1", "rho_w2", "rho_b2", "rho_w3", "rho_b3")]
    h = phi_rho_numpy(x, *args, *rargs)
    r = compact_ranks(inputs["event_ids"])
    pooled = np.zeros((n_events, h.shape[1]), np.float32)
    np.add.at(pooled, r, h)
    return rho_numpy(pooled, *rargs).astype(np.float32)


def kernel(**inputs):
    cfg = FULL_CFG
    T = np.asarray(inputs["x"]).shape[0]
    n_events = 100_000
    if T != cfg.n_cores * cfg.T_core:
        return _numpy_fallback(inputs, n_events)
    try:
        in_maps, metas, r, D = prepare_in_maps(inputs, cfg)
    except (ScheduleOverflow, AssertionError):
        return _numpy_fallback(inputs, n_events)
    nc = _get_program(cfg)
    global _LAST_RES
    trace = TRACE and _install_ntff_hook()
    res = run_bass_kernel_spmd(nc, in_maps, core_ids=list(range(cfg.n_cores)),
                               trace=trace)
    _LAST_RES = res
    return assemble_output(res.results, metas, r, D, inputs, cfg, n_events)


# revision 25
# speedup vs baseline: 1.1794x; 1.0617x over previous
"""DeepSet (segment_reduce) Trainium2 kernel.

Model (per reference):
    h  = relu(relu(x @ w1 + b1) @ w2 + b2)          # phi, per track
    pooled[e] = sum_{t in event e} h[t]             # segment sum (sorted ids)
    y  = sigmoid(relu(relu(pooled@rw1+rb1)@rw2+rb2)@rw3+rb3)   # rho, per event

Strategy (8 NeuronCores, SPMD single program):
  - Shard tracks in fixed 250k blocks per core (NOT event aligned); the few
    boundary events that straddle cores are recomputed exactly on the host
    (tiny) and patched into the output.
  - Host reorders x into a transposed interleaved layout xt4 so the device
    streams it with perfectly contiguous DMA and feeds the PE directly
    (contraction dim on partitions, no on-device transposes for phi).
  - phi: mm1 (f32r, weights stationary, 4x row-tiled K=32) -> relu (ACT, bias)
    -> h1T bf16; mm2 (h1T chunks as stationary, K=64 2x row-tiled) -> h2
    natural [track, latent] -> relu (DVE) -> bf16.
  - pooling: per 128-track tile, a data-dependent onehot [128 tracks x 64
    event-slots] built on DVE from host-precomputed per-track slot offsets,
    then matmul accumulates into rotating PSUM window banks. Event ids are
    renumbered by the host into padded "slots" so the whole schedule (window
    indices, start flags, flushes) is a pure function of tile index ->
    identical instruction stream on every core.
  - rho: after pooling, PE transposes pooled blocks, small matmuls + ACT.
  - Boundary events / event ids that never appear are patched on host.
"""

import math
import os
from contextlib import ExitStack

import numpy as np
import ml_dtypes

import concourse.bass as bass
import concourse.tile as tile
from concourse import bacc, mybir
from concourse.bass_utils import run_bass_kernel_spmd

BF16 = ml_dtypes.bfloat16
FP32 = np.float32
AF = mybir.ActivationFunctionType
ALU = mybir.AluOpType
dt = mybir.dt


class Cfg:
    def __init__(self, n_cores=8, tracks_per_core=250_000, tiles_per_window=4,
                 mm1_dtype="bfloat16"):
        self.n_cores = n_cores
        self.F = 32           # input features
        self.L = 64           # latent width (phi hidden and output width)
        self.RH = 128         # rho hidden width
        self.T_core = tracks_per_core
        self.G = 4096         # tracks per DMA super-tile
        # padded tracks per core (multiple of G)
        self.TPAD = ((tracks_per_core + 128 + self.G - 1) // self.G) * self.G
        self.NT = self.TPAD // 128          # 128-track tiles per core
        self.TPW = tiles_per_window         # tiles per 32-slot window
        self.NWIN = self.NT // self.TPW + 2  # windows
        self.NBANK = (self.NWIN + 3) // 4   # 128-slot psum banks
        self.SPAD = self.NBANK * 128        # padded slot count
        self.mm1_dtype = mm1_dtype

    def __repr__(self):
        return (f"Cfg(cores={self.n_cores},TPAD={self.TPAD},NT={self.NT},"
                f"TPW={self.TPW},SPAD={self.SPAD},mm1={self.mm1_dtype})")


FULL_CFG = Cfg()


# --------------------------------------------------------------------------
# Host-side planning
# --------------------------------------------------------------------------

class ScheduleOverflow(Exception):
    pass


def compact_ranks(event_ids):
    ev = np.asarray(event_ids)
    change = (ev[1:] != ev[:-1]).astype(np.int64)
    r = np.concatenate([[0], np.cumsum(change)]).astype(np.int64)
    return r


def plan_core(r_local, cfg):
    """Assign padded slots to local events and build per-track seg values.

    r_local: int64 [Tc] local event ranks (0-based, non-decreasing) for the
             tracks owned by this core (boundary events included; their
             outputs are discarded later).
    Returns (seg_rel bf16 [128, NT], slot_of_event int64 [n_local_events]).
    """
    Tc = len(r_local)
    NT, TPW = cfg.NT, cfg.TPW
    n_ev = int(r_local[-1]) + 1 if Tc else 0
    first_track = np.searchsorted(r_local, np.arange(n_ev), side="left")
    last_track = np.searchsorted(r_local, np.arange(n_ev), side="right") - 1
    first_tile = first_track // 128
    last_tile = last_track // 128

    # slot assignment: slot[e] = max(counter, 32*(first_tile//TPW),
    #                                32*(last_tile//TPW))
    slot = np.zeros(n_ev, dtype=np.int64)
    counter = 0
    base_first = 32 * (first_tile // TPW)
    base_last = 32 * (last_tile // TPW)
    lo = np.maximum(base_first, base_last)
    for e in range(n_ev):
        counter = max(counter, lo[e])
        slot[e] = counter
        counter += 1
    # validity: for every tile i and event e in it:
    #   0 <= slot[e] - 32*(i//TPW) < 64
    # worst cases are at first_tile (largest rel) and last_tile (smallest rel)
    rel_hi = slot - base_first
    if rel_hi.max(initial=0) >= 64:
        raise ScheduleOverflow(f"max rel {rel_hi.max()} >= 64")
    if (slot - base_last).min(initial=0) < 0:
        raise ScheduleOverflow("negative rel")
    if slot.max(initial=0) >= cfg.SPAD:
        raise ScheduleOverflow("slot overflow")

    # per-track values: slot[r] - 32*(tile//TPW)
    tiles = np.arange(cfg.TPAD) // 128
    seg = np.full(cfg.TPAD, -512.0, dtype=np.float64)
    seg[:Tc] = slot[r_local] - 32.0 * (tiles[:Tc] // TPW)
    segT = seg.reshape(NT, 128).T.astype(BF16)  # [128, NT] col i = tile i
    # bf16 must represent all values exactly in the comparison-critical range
    return np.ascontiguousarray(segT), slot


def make_xt4(x_pad, cfg):
    """[TPAD, F] f32 -> [128, TPAD//4] interleaved transposed layout.

    track t = 4096 g + 1024 b + j maps to partition 32 b + f, column
    1024 g + j.
    """
    G = cfg.G
    ng = cfg.TPAD // G
    xt = x_pad.reshape(ng, 4, G // 4, cfg.F).transpose(1, 3, 0, 2)
    return np.ascontiguousarray(xt.reshape(128, -1))


def emission_order(cfg):
    """Tile indices in device processing order (must match build_program)."""
    order = []
    for g in range(cfg.TPAD // cfg.G):
        for hab in range(2):
            i0 = 32 * g + 16 * hab
            for t2 in range(2):
                for m in range(8):
                    order.append(i0 + 8 * (m % 2) + 4 * t2 + m // 2)
    return order


def phi_rho_numpy(x, w1, b1, w2, b2, rw1, rb1, rw2, rb2, rw3, rb3):
    h = np.maximum(x @ w1 + b1, 0.0)
    h = np.maximum(h @ w2 + b2, 0.0)
    return h


def rho_numpy(pooled, rw1, rb1, rw2, rb2, rw3, rb3):
    r = np.maximum(pooled @ rw1 + rb1, 0.0)
    r = np.maximum(r @ rw2 + rb2, 0.0)
    z = r @ rw3 + rb3
    return 1.0 / (1.0 + np.exp(-z))


# --------------------------------------------------------------------------
# Device program
# --------------------------------------------------------------------------

def build_program(cfg, relu2_engine="vector", onehot_engine="vector"):
    nc = bacc.Bacc("TRN2", target_bir_lowering=False, debug=False,
                   enable_asserts=False, num_devices=cfg.n_cores)
    F, L, RH = cfg.F, cfg.L, cfg.RH
    NT, TPW = cfg.NT, cfg.TPW
    mm1dt = getattr(dt, cfg.mm1_dtype)

    xt4_d = nc.dram_tensor("xt4", [128, cfg.TPAD // 4], mm1dt,
                           kind="ExternalInput").ap()
    segT_d = nc.dram_tensor("segT", [128, NT], dt.bfloat16,
                            kind="ExternalInput").ap()
    w1_d = nc.dram_tensor("w1blk", [128, 128], mm1dt,
                          kind="ExternalInput").ap()
    b1_d = nc.dram_tensor("b1rep", [128, 1], dt.float32,
                          kind="ExternalInput").ap()
    w2_d = nc.dram_tensor("w2stk", [128, 128], dt.bfloat16,
                          kind="ExternalInput").ap()
    rw1_d = nc.dram_tensor("rw1rep", [64, RH], dt.float32r,
                           kind="ExternalInput").ap()
    rb1_d = nc.dram_tensor("rb1", [128, 1], dt.float32,
                           kind="ExternalInput").ap()
    rw2_d = nc.dram_tensor("rw2", [128, L], dt.float32r,
                           kind="ExternalInput").ap()
    rb2_d = nc.dram_tensor("rb2", [64, 1], dt.float32,
                           kind="ExternalInput").ap()
    rw3_d = nc.dram_tensor("rw3", [64, 1], dt.float32r,
                           kind="ExternalInput").ap()
    rb3_d = nc.dram_tensor("rb3", [1, 1], dt.float32,
                           kind="ExternalInput").ap()
    ident_d = nc.dram_tensor("ident", [128, 128], dt.float32,
                             kind="ExternalInput").ap()
    y_d = nc.dram_tensor("y", [1, cfg.SPAD], dt.float32,
                         kind="ExternalOutput").ap()

    with tile.TileContext(nc) as tc, ExitStack() as ctx:
        const = ctx.enter_context(tc.tile_pool(name="const", bufs=1))
        w1_s = const.tile([128, 128], mm1dt, tag="w1")
        nc.sync.dma_start(w1_s[:], w1_d)
        b1_s = const.tile([128, 1], dt.float32, tag="b1")
        nc.sync.dma_start(b1_s[:], b1_d)
        w2_s = const.tile([128, 128], dt.bfloat16, tag="w2")
        nc.sync.dma_start(w2_s[:], w2_d)
        seg_s = const.tile([128, NT], dt.bfloat16, tag="seg")
        nc.sync.dma_start(seg_s[:], segT_d)
        iota_i = const.tile([128, 64], dt.int32, tag="iotai")
        nc.gpsimd.iota(iota_i[:], pattern=[[1, 64]], base=0,
                       channel_multiplier=0)
        iota_s = const.tile([128, 64], dt.bfloat16, tag="iotab")
        nc.vector.tensor_copy(iota_s[:], iota_i[:])

        pooled_pool = ctx.enter_context(tc.tile_pool(name="pooled", bufs=1))
        pooled = pooled_pool.tile([128, cfg.NBANK * L], dt.float32)

        relu2_eng = nc.scalar if relu2_engine == "scalar" else getattr(nc, relu2_engine)

        # ---------------- main loop ----------------
        started = set()     # windows whose psum quarter got start=True
        bank_tiles = {}     # bank index -> psum tile object
        with (
            tc.tile_pool(name="xt", bufs=6) as xt_pool,
            tc.tile_pool(name="p1", bufs=2, space="PSUM") as p1_pool,
            tc.tile_pool(name="h1", bufs=6) as h1_pool,
            tc.tile_pool(name="p2", bufs=2, space="PSUM") as p2_pool,
            tc.tile_pool(name="h2", bufs=6) as h2_pool,
            tc.tile_pool(name="oh", bufs=6) as oh_pool,
            tc.tile_pool(name="p3", bufs=2, space="PSUM") as p3_pool,
        ):
            def flush_bank(b):
                bt = bank_tiles.pop(b)
                nc.vector.tensor_copy(pooled[:, L * b:L * (b + 1)], bt[:])

            def get_bank(b):
                if b not in bank_tiles:
                    bank_tiles[b] = p3_pool.tile([128, L], dt.float32,
                                                 tag="bank", name=f"bank{b}")
                return bank_tiles[b]

            def mm3_pass(i, oh_col_ap, h2_ap):
                # windows w1=i//TPW (onehot cols 0:32) and w1+1 (cols 32:64)
                w1 = i // TPW
                q1 = w1 % 4
                s1, s2 = w1 not in started, (w1 + 1) not in started
                if q1 in (0, 2) and s1 == s2:
                    bt = get_bank(w1 // 4)
                    started.add(w1)
                    started.add(w1 + 1)
                    nc.tensor.matmul(
                        bt[32 * q1:32 * q1 + 64, :], oh_col_ap, h2_ap,
                        start=s1, stop=True, skip_group_check=True,
                        tile_position=(0, 32 * q1))
                    return
                for p, w in enumerate((w1, w1 + 1)):
                    b, q = w // 4, w % 4
                    bt = get_bank(b)
                    first = w not in started
                    started.add(w)
                    nc.tensor.matmul(
                        bt[32 * q:32 * (q + 1), :],
                        oh_col_ap[:, 32 * p:32 * (p + 1)],
                        h2_ap,
                        start=first, stop=True, skip_group_check=True,
                        tile_position=(0, 32 * q))

            pos = 0   # emission position == seg column index
            for g in range(cfg.TPAD // cfg.G):
                xt_t = xt_pool.tile([128, 1024], mm1dt, tag="xt")
                nc.sync.dma_start(xt_t[:], xt4_d[:, 1024 * g:1024 * (g + 1)])
                for hab in range(2):
                    p1 = p1_pool.tile([128, 1024], dt.float32, tag="p1")
                    for h in range(2):
                        for cg in range(2):
                            nc.tensor.matmul(
                                p1[64 * cg:64 * (cg + 1),
                                   512 * h:512 * (h + 1)],
                                w1_s[64 * hab:64 * (hab + 1),
                                     64 * cg:64 * (cg + 1)],
                                xt_t[64 * hab:64 * (hab + 1),
                                     512 * h:512 * (h + 1)],
                                start=True, stop=True,
                                tile_position=(64 * hab, 64 * cg))
                    h1 = h1_pool.tile([128, 1024], dt.bfloat16, tag="h1")
                    nc.scalar.activation(h1[:], p1[:], AF.Relu, bias=b1_s[:])
                    i0 = 32 * g + 16 * hab
                    for t2 in range(2):
                        p2 = p2_pool.tile([128, 512], dt.float32, tag="p2")
                        for m4 in range(4):
                            j = 4 * t2 + m4
                            nc.tensor.matmul(
                                p2[:, 128 * m4:128 * (m4 + 1)],
                                h1[:, 128 * j:128 * (j + 1)],
                                w2_s[:],
                                start=True, stop=True)
                        h2 = h2_pool.tile([128, 512], dt.bfloat16, tag="h2")
                        if (pos // 8) % 2 == 0:
                            nc.vector.tensor_scalar_max(h2[:], p2[:], 0.0)
                        else:
                            nc.scalar.activation(h2[:], p2[:], AF.Relu)
                        oh = oh_pool.tile([128, 512], dt.bfloat16, tag="oh")
                        seg_ap = (seg_s[:, pos:pos + 8]
                                  .unsqueeze(2).to_broadcast([128, 8, 64]))
                        iota_ap = (iota_s[:].unsqueeze(1)
                                   .to_broadcast([128, 8, 64]))
                        oh_ap = oh[:].rearrange("p (a b) -> p a b", b=64)
                        nc.vector.tensor_tensor(
                            oh_ap, iota_ap, seg_ap, ALU.is_equal)
                        for m in range(8):
                            i = i0 + 8 * (m % 2) + 4 * t2 + m // 2
                            mm3_pass(i, oh[:, 64 * m:64 * (m + 1)],
                                     h2[:, 64 * m:64 * (m + 1)])
                        pos += 8
                    bid = 2 * g + hab
                    if bid in bank_tiles:
                        flush_bank(bid)
            # tail: give untouched windows a start matmul so psum is defined,
            # then flush remaining banks.  Handle live banks first, then any
            # completely-untouched banks one at a time (alloc -> fill ->
            # flush) so the 2-slot psum pool never holds >2 live banks.
            zt = oh_pool.tile([128, 512], dt.bfloat16, tag="oh")
            nc.vector.memset(zt[:, 0:64], 0.0)
            zh = h2_pool.tile([128, 512], dt.bfloat16, tag="h2")
            nc.vector.memset(zh[:, 0:64], 0.0)

            def pad_window(w):
                b, q = w // 4, w % 4
                nc.tensor.matmul(bank_tiles[b][32 * q:32 * (q + 1), :],
                                 zt[:, 0:32], zh[:, 0:64],
                                 start=True, stop=True, skip_group_check=True,
                                 tile_position=(0, 32 * q))

            for b in sorted(bank_tiles):
                for q in range(4):
                    if 4 * b + q not in started:
                        pad_window(4 * b + q)
                flush_bank(b)
            for b in range(cfg.NBANK):
                if b in bank_tiles or not any(
                        4 * b + q not in started for q in range(4)):
                    continue
                bank_tiles[b] = p3_pool.tile([128, L], dt.float32, tag="bank",
                                             name=f"bank{b}")
                for q in range(4):
                    pad_window(4 * b + q)
                flush_bank(b)

        # ---------------- rho (f32r path for accuracy) ----------------
        f32r = dt.float32r
        rho_const = ctx.enter_context(tc.tile_pool(name="rhoc", bufs=1))
        rw1_s = rho_const.tile([64, RH], f32r, tag="rw1")
        nc.sync.dma_start(rw1_s[:], rw1_d)
        rb1_s = rho_const.tile([128, 1], dt.float32, tag="rb1")
        nc.sync.dma_start(rb1_s[:], rb1_d)
        rw2_s = rho_const.tile([128, L], f32r, tag="rw2")
        nc.sync.dma_start(rw2_s[:], rw2_d)
        rb2_s = rho_const.tile([64, 1], dt.float32, tag="rb2")
        nc.sync.dma_start(rb2_s[:], rb2_d)
        rw3_s = rho_const.tile([64, 1], f32r, tag="rw3")
        nc.sync.dma_start(rw3_s[:], rw3_d)
        rb3_s = rho_const.tile([1, 1], dt.float32, tag="rb3")
        nc.sync.dma_start(rb3_s[:], rb3_d)
        id_s = rho_const.tile([128, 128], dt.float32, tag="ident")
        nc.sync.dma_start(id_s[:], ident_d)

        blocks = []
        b0 = 0
        while b0 < cfg.NBANK:
            nb = min(4, cfg.NBANK - b0)
            blocks.append((b0, nb))
            b0 += nb
        with (
            tc.tile_pool(name="tp", bufs=2, space="PSUM") as tp_pool,
            tc.tile_pool(name="ptsb", bufs=2) as pt_pool,
            tc.tile_pool(name="r1p", bufs=2, space="PSUM") as r1p_pool,
            tc.tile_pool(name="r1s", bufs=2) as r1s_pool,
            tc.tile_pool(name="r2p", bufs=2, space="PSUM") as r2p_pool,
            tc.tile_pool(name="r2s", bufs=2) as r2s_pool,
            tc.tile_pool(name="yp", bufs=2, space="PSUM") as yp_pool,
            tc.tile_pool(name="ys", bufs=2) as ys_pool,
        ):
            for (b0, nb) in blocks:
                S = 128 * nb
                tp = tp_pool.tile([64, 512], dt.float32, tag="tp")
                for j in range(nb):
                    nc.tensor.transpose(
                        tp[:, 128 * j:128 * (j + 1)],
                        pooled[:, L * (b0 + j):L * (b0 + j + 1)],
                        id_s[:])
                pt = pt_pool.tile([64, 512], f32r, tag="pt")
                nc.vector.tensor_copy(pt[:, 0:S], tp[:, 0:S])
                r1p = r1p_pool.tile([128, 512], dt.float32, tag="r1p")
                nc.tensor.matmul(r1p[:, 0:S], rw1_s[:], pt[:, 0:S],
                                 start=True, stop=True)
                r1s = r1s_pool.tile([128, 512], f32r, tag="r1s")
                nc.scalar.activation(r1s[:, 0:S], r1p[:, 0:S], AF.Relu,
                                     bias=rb1_s[:])
                r2p = r2p_pool.tile([64, 512], dt.float32, tag="r2p")
                nc.tensor.matmul(r2p[:, 0:S], rw2_s[:], r1s[:, 0:S],
                                 start=True, stop=True)
                r2s = r2s_pool.tile([64, 512], f32r, tag="r2s")
                nc.scalar.activation(r2s[:, 0:S], r2p[:, 0:S], AF.Relu,
                                     bias=rb2_s[:])
                yp = yp_pool.tile([1, 512], dt.float32, tag="yp")
                nc.tensor.matmul(yp[:, 0:S], rw3_s[:], r2s[:, 0:S],
                                 start=True, stop=True)
                ys = ys_pool.tile([1, 512], dt.float32, tag="ys")
                nc.vector.tensor_copy(ys[:, 0:S], yp[:, 0:S])
                nc.sync.dma_start(y_d[:, 128 * b0:128 * b0 + S], ys[:, 0:S])

    nc.compile()
    return nc


# --------------------------------------------------------------------------
# kernel() entry point
# --------------------------------------------------------------------------

_PROG_CACHE = {}
TRACE = False
_LAST_RES = None


def _install_ntff_hook():
    """Register the axon NTFF profiling hook if the image lacks
    antenv.axon_hooks (needed for run_bass_kernel_spmd(trace=True))."""
    import sys, types
    try:
        from antenv.axon_hooks import get_axon_ntff_profile_hook  # noqa: F401
        return True
    except ImportError:
        pass
    try:
        from trn_agent_boot.trn_boot import _ntff_profile_via_ctypes
        hook = _ntff_profile_via_ctypes("/opt/axon/libaxon_pjrt.so")
        if hook is None:
            return False
        mod = types.ModuleType("antenv.axon_hooks")
        mod.get_axon_ntff_profile_hook = lambda: hook
        mod.set_axon_ntff_profile_hook = lambda h: None
        sys.modules["antenv.axon_hooks"] = mod
        return True
    except Exception:
        return False


def _get_program(cfg, **kw):
    key = (repr(cfg), tuple(sorted(kw.items())))
    if key not in _PROG_CACHE:
        _PROG_CACHE[key] = build_program(cfg, **kw)
    return _PROG_CACHE[key]


def prepare_in_maps(inputs, cfg):
    x = np.asarray(inputs["x"], np.float32)
    ev = np.asarray(inputs["event_ids"])
    w1 = np.asarray(inputs["phi_w1"], np.float32)
    b1 = np.asarray(inputs["phi_b1"], np.float32)
    w2 = np.asarray(inputs["phi_w2"], np.float32)
    b2 = np.asarray(inputs["phi_b2"], np.float32)
    assert np.all(b2 == 0.0), "phi_b2 != 0 unsupported fast path"
    T = x.shape[0]
    r = compact_ranks(ev)
    D = int(r[-1]) + 1

    mm1_np = BF16 if cfg.mm1_dtype == "bfloat16" else np.float32
    blk = np.zeros((64, 128), np.float32)
    blk[0:32, 0:64] = w1
    blk[32:64, 64:128] = w1
    w1blk = np.vstack([blk, blk]).astype(mm1_np)
    w2stk = np.zeros((128, 128), np.float32)
    w2stk[0:64, 0:64] = w2
    w2stk[64:128, 64:128] = w2
    w2stk = w2stk.astype(BF16)
    b1rep = np.tile(b1.reshape(-1), 2).reshape(128, 1).astype(np.float32)
    rw1rep = np.asarray(inputs["rho_w1"], np.float32)
    rb1 = np.asarray(inputs["rho_b1"], np.float32).reshape(128, 1)
    rw2 = np.asarray(inputs["rho_w2"], np.float32)
    rb2 = np.asarray(inputs["rho_b2"], np.float32).reshape(64, 1)
    rw3 = np.asarray(inputs["rho_w3"], np.float32)
    rb3 = np.asarray(inputs["rho_b3"], np.float32).reshape(1, 1)
    ident = np.eye(128, dtype=np.float32)

    in_maps, metas = [], []
    for c in range(cfg.n_cores):
        s, e = c * cfg.T_core, min((c + 1) * cfg.T_core, T)
        r_loc_g = r[s:e]
        e0 = int(r_loc_g[0])
        r_loc = (r_loc_g - e0).astype(np.int64)
        segT, slot = plan_core(r_loc, cfg)
        segT = np.ascontiguousarray(segT[:, emission_order(cfg)])
        xp = np.zeros((cfg.TPAD, cfg.F), np.float32)
        xp[:e - s] = x[s:e]
        in_maps.append({
            "xt4": make_xt4(xp, cfg).astype(mm1_np),
            "segT": segT,
            "w1blk": w1blk, "b1rep": b1rep, "w2stk": w2stk,
            "rw1rep": rw1rep, "rb1": rb1, "rw2": rw2, "rb2": rb2,
            "rw3": rw3, "rb3": rb3, "ident": ident,
        })
        # events fully owned by this core (not straddling boundary)
        n_ev = int(r_loc[-1]) + 1
        own_lo = 0 if s == 0 else (1 if r[s - 1] == r[s] else 0)
        own_hi = n_ev if e == T else (n_ev - 1 if r[e - 1] == r[e] else n_ev)
        metas.append(dict(e0=e0, n_ev=n_ev, own_lo=own_lo, own_hi=own_hi,
                          slot=slot))
    return in_maps, metas, r, D


def assemble_output(results, metas, r, D, inputs, cfg, n_events):
    x = np.asarray(inputs["x"], np.float32)
    args = [np.asarray(inputs[k], np.float32) for k in
            ("phi_w1", "phi_b1", "phi_w2", "phi_b2")]
    rargs = [np.asarray(inputs[k], np.float32) for k in
             ("rho_w1", "rho_b1", "rho_w2", "rho_b2", "rho_w3", "rho_b3")]
    y = np.empty(n_events, np.float32)
    if D < n_events:
        y[D:] = rho_numpy(np.zeros((1, cfg.L), np.float32), *rargs)[0, 0]
    covered = np.zeros(D, bool)
    rb3s = float(np.asarray(inputs["rho_b3"]).reshape(-1)[0])
    for c, (res, m) in enumerate(zip(results, metas)):
        z = res["y"].reshape(-1).astype(np.float64) + rb3s
        yc = (1.0 / (1.0 + np.exp(-z))).astype(np.float32)
        sl = m["slot"][m["own_lo"]:m["own_hi"]]
        ge = m["e0"] + np.arange(m["own_lo"], m["own_hi"])
        y[ge] = yc[sl]
        covered[ge] = True
    # patch uncovered (boundary) events exactly on host
    missing = np.nonzero(~covered)[0]
    if len(missing):
        starts = np.searchsorted(r, missing, side="left")
        ends = np.searchsorted(r, missing, side="right")
        for e, st, en in zip(missing, starts, ends):
            h = phi_rho_numpy(x[st:en], *args, *rargs)
            pooled = h.sum(0, keepdims=True)
            y[e] = rho_numpy(pooled, *rargs)[0, 0]
    return y.reshape(-1, 1)


def _numpy_fallback(inputs, n_events):
    """Reference-exact host computation (used only if the input does not fit
    the compiled schedule)."""
    x = np.asarray(inputs["x"], np.float32)
    args = [np.asarray(inputs[k], np.float32) for k in
            ("phi_w1", "phi_b1", "phi_w2", "phi_b2")]
    rargs = [np.asarray(inputs[k], np.float32) for k in
             ("rho_w1", "rho_b1", "rho_w2", "rho_b2", "rho_w3", "rho_b3")]
    h = phi_rho_numpy(x, *args, *rargs)
    r = compact_ranks(inputs["event_ids"])
    pooled = np.zeros((n_events, h.shape[1]), np.float32)
    np.add.at(pooled, r, h)
    return rho_numpy(pooled, *rargs).astype(np.float32)


def kernel(**inputs):
    cfg = FULL_CFG
    T = np.asarray(inputs["x"]).shape[0]
    n_events = 100_000
    if T != cfg.n_cores * cfg.T_core:
        return _numpy_fallback(inputs, n_events)
    try:
        in_maps, metas, r, D = prepare_in_maps(inputs, cfg)
    except (ScheduleOverflow, AssertionError):
        return _numpy_fallback(inputs, n_events)
    nc = _get_program(cfg)
    global _LAST_RES
    trace = TRACE and _install_ntff_hook()
    res = run_bass_kernel_spmd(nc, in_maps, core_ids=list(range(cfg.n_cores)),
                               trace=trace)
    _LAST_RES = res
    return assemble_output(res.results, metas, r, D, inputs, cfg, n_events)
